# revision 1
# baseline (speedup 1.0000x reference)
"""Trainium2 Bass kernel for nn_Beltrami (retrieval_knn).

Per-core (batch-parallel over 8 cores): fc (f32r hi/lo-split matmuls) ->
normalize pos -> cosine sim (3-term f32r split, fp32 accuracy at 1 cyc/row) ->
top-32 threshold via 32-chunk max8 + 4-round refine -> exact is_ge mask * exp
-> A@[feat|1] fp16 matmul via blocked transpose-DMA -> softmax-normalized out.
"""
import sys
import numpy as np

sys.path.insert(0, "/opt/trn_rl_repo")

B, N, C, K = 8, 4096, 256, 32
NT = N // 128          # 32 query tiles of 128 rows
NEG = -1.0e30

_CACHE = {}


def _build(reps=1):
    from contextlib import ExitStack
    import concourse.bass as bass
    import concourse.bacc as bacc
    import concourse.tile as tile
    from concourse import mybir

    f32 = mybir.dt.float32
    f32r = mybir.dt.float32r
    f16 = mybir.dt.float16
    AF = mybir.ActivationFunctionType
    Alu = mybir.AluOpType

    nc = bacc.Bacc("TRN2", target_bir_lowering=False, debug=False, num_devices=8)

    xT_in = nc.declare_dram_parameter("xT", [C, N], f32, isOutput=False)
    wT_in = nc.declare_dram_parameter("wT", [C, 2 * C], f32, isOutput=False)
    bf_in = nc.declare_dram_parameter("bf", [1, 2 * C], f32, isOutput=False)
    bp_in = nc.declare_dram_parameter("bp", [128, 2], f32, isOutput=False)
    id_in = nc.declare_dram_parameter("ident", [128, 128], f32, isOutput=False)
    out_p = nc.declare_dram_parameter("out", [N, C], f32, isOutput=True)
    a_drams = [nc.dram_tensor(f"a_scratch{i}", [128, N], f16) for i in range(3)]
    s_dram = nc.dram_tensor("s_scratch", [NT, 128], f32)

    with tile.TileContext(nc) as tc, ExitStack() as ctx:
        # ---------------- persistent pools ----------------
        persist = ctx.enter_context(tc.tile_pool(name="persist", bufs=1))
        featx_pool = ctx.enter_context(tc.tile_pool(name="featx", bufs=NT))

        post_hi = [persist.tile([128, N], f32r, tag=f"post_hi{ct}", name=f"post_hi{ct}") for ct in range(2)]
        post_lo = [persist.tile([128, N], f32r, tag=f"post_lo{ct}", name=f"post_lo{ct}") for ct in range(2)]
        featx = [featx_pool.tile([128, C + 2], f16, tag="featx", name=f"featx{i}") for i in range(NT)]
        nrm2 = persist.tile([128, NT], f32, tag="nrm2")
        s_til = persist.tile([128, NT], f32, tag="s_til")

        # ---------------- startup: fc + normalize ----------------
        with ExitStack() as sctx:
            sb = sctx.enter_context(tc.tile_pool(name="start_sb", bufs=1))
            ps_fc = sctx.enter_context(tc.tile_pool(name="ps_fc", bufs=3, space="PSUM"))
            ps_pp = sctx.enter_context(tc.tile_pool(name="ps_pp", bufs=3, space="PSUM"))
            ps_tp = sctx.enter_context(tc.tile_pool(name="ps_tp", bufs=1, space="PSUM"))

            xstage_pool = sctx.enter_context(tc.tile_pool(name="xstage_pool", bufs=3))
            xt_r = [sb.tile([128, N], f32r, tag=f"xt_r{ct}", name=f"xt_r{ct}") for ct in range(2)]
            wt = [sb.tile([128, 2 * C], f32, tag=f"wt{ct}", name=f"wt{ct}") for ct in range(2)]
            wt_r = [sb.tile([128, C], f32r, tag=f"wt_r{ct}", name=f"wt_r{ct}") for ct in range(2)]
            xt_lo = [sb.tile([128, N], f32r, tag=f"xt_lo{ct}", name=f"xt_lo{ct}") for ct in range(2)]
            wph = [sb.tile([128, C], f32r, tag=f"wph{ct}", name=f"wph{ct}") for ct in range(2)]
            wpl = [sb.tile([128, C], f32r, tag=f"wpl{ct}", name=f"wpl{ct}") for ct in range(2)]
            bf1_r = sb.tile([1, 2 * C], f32r, tag="bf1_r")
            ones1_r = sb.tile([1, 128], f32r, tag="ones1_r")
            bf1 = sb.tile([1, 2 * C], f32, tag="bf1")
            bp = sb.tile([128, 2], f32, tag="bp")
            ident = sb.tile([128, 128], f32, tag="ident")
            ones1 = sb.tile([1, 128], f32, tag="ones1")
            scrap = sb.tile([128, C], f16, tag="scrap")
            post_raw = [sb.tile([128, N], f32, tag=f"post_raw{ct}", name=f"post_raw{ct}") for ct in range(2)]

            for ct in range(2):
                nc.sync.dma_start(wt[ct][:], wT_in[ct * 128:(ct + 1) * 128, :])
            nc.sync.dma_start(bf1[:], bf_in[:])
            nc.sync.dma_start(bp[:], bp_in[:])
            nc.sync.dma_start(ident[:], id_in[:])
            nc.vector.memset(ones1[:], 1.0)
            for ct in range(2):
                nc.vector.tensor_copy(wt_r[ct][:], wt[ct][:, 0:C])
                nc.vector.tensor_copy(wph[ct][:], wt[ct][:, C:2 * C])
                nc.vector.tensor_tensor(wpl[ct][:], wt[ct][:, C:2 * C],
                                        wph[ct][:], op=Alu.subtract)
            nc.vector.tensor_copy(bf1_r[:], bf1[:])
            nc.vector.tensor_copy(ones1_r[:], ones1[:])

            # staging + fc interleaved per 512-chunk so PE stays dense
            def stage_chunk(ch):
                cs = slice(ch * 512, (ch + 1) * 512)
                for ct in range(2):
                    xstage = xstage_pool.tile([128, 512], f32, tag="xstage",
                                              name=f"xstage{ct}_{ch}")
                    nc.sync.dma_start(xstage[:], xT_in[ct * 128:(ct + 1) * 128, cs])
                    nc.vector.tensor_copy(xt_r[ct][:, cs], xstage[:])
                    nc.vector.tensor_tensor(xt_lo[ct][:, cs], xstage[:],
                                            xt_r[ct][:, cs], op=Alu.subtract)

            def posT_chunk(dt, ch):
                pp = ps_pp.tile([128, 512], f32, tag="pp", name=f"pp{dt}_{ch}")
                ds_ = slice(dt * 128, (dt + 1) * 128)
                cs_ = slice(ch * 512, (ch + 1) * 512)
                for ci, (lh, rh) in enumerate(
                        [(wph[0], xt_r[0]), (wph[0], xt_lo[0]), (wpl[0], xt_r[0]),
                         (wph[1], xt_r[1]), (wph[1], xt_lo[1]), (wpl[1], xt_r[1])]):
                    nc.tensor.matmul(pp[:], lh[:, ds_], rh[:, cs_],
                                     start=(ci == 0), stop=(ci == 5))
                nc.scalar.activation(
                    post_raw[dt][:, ch * 512:(ch + 1) * 512], pp[:],
                    AF.Identity, bias=bp[:, dt:dt + 1])

            for ch in range(8):
                stage_chunk(ch)
            for nt in range(NT):
                fc = ps_fc.tile([128, 2 * C], f32, tag="fc")
                ns = slice(nt * 128, (nt + 1) * 128)
                nc.tensor.matmul(fc[:, 0:C], xt_r[0][:, ns], wt_r[0][:],
                                 start=True, stop=False)
                nc.tensor.matmul(fc[:, 0:C], xt_r[1][:, ns], wt_r[1][:],
                                 start=False, stop=False)
                nc.tensor.matmul(fc[:, 0:C], ones1_r[:], bf1_r[:, 0:C],
                                 start=False, stop=True)
                for ci, (lh, rh) in enumerate(
                        [(xt_r[0], wph[0]), (xt_r[0], wpl[0]), (xt_lo[0], wph[0]),
                         (xt_r[1], wph[1]), (xt_r[1], wpl[1]), (xt_lo[1], wph[1])]):
                    nc.tensor.matmul(fc[:, C:2 * C], lh[:, ns], rh[:],
                                     start=(ci == 0), stop=False)
                nc.tensor.matmul(fc[:, C:2 * C], ones1_r[:], bf1_r[:, C:2 * C],
                                 start=False, stop=True)
                nc.vector.tensor_copy(featx[nt][:, 0:C], fc[:, 0:C])
                nc.gpsimd.memset(featx[nt][:, C:C + 1], 1.0)
                nc.gpsimd.memset(featx[nt][:, C + 1:C + 2], 0.0)
                nc.scalar.activation(scrap[:], fc[:, C:2 * C], AF.Square,
                                     accum_out=nrm2[:, nt:nt + 1])


            for ch in range(8):
                posT_chunk(0, ch)
                posT_chunk(1, ch)

            # rsqrt of norms with two Newton steps
            r0 = sb.tile([128, NT], f32, tag="r0")
            u = sb.tile([128, NT], f32, tag="u")
            nc.vector.reciprocal(r0[:], nrm2[:])
            nc.scalar.activation(s_til[:], r0[:], AF.Sqrt)
            for _ in range(2):
                nc.vector.tensor_tensor(u[:], s_til[:], s_til[:], op=Alu.mult)
                nc.vector.tensor_tensor(u[:], u[:], nrm2[:], op=Alu.mult)
                nc.vector.tensor_scalar(u[:], u[:], -0.5, scalar2=1.5,
                                        op0=Alu.mult, op1=Alu.add)
                nc.vector.tensor_tensor(s_til[:], s_til[:], u[:], op=Alu.mult)

            # transpose s [128, NT] -> [NT, 128], bounce via DRAM, broadcast-load
            st_ps = ps_tp.tile([NT, 128], f32, tag="st_ps")
            nc.tensor.transpose(st_ps[:], s_til[:], ident[:])
            stt = sb.tile([NT, 128], f32, tag="stt")
            nc.vector.tensor_copy(stt[:], st_ps[:])
            nc.sync.dma_start(s_dram[:], stt[:])

            for ch in range(8):
                cs = slice(ch * 512, (ch + 1) * 512)
                sbc = xstage_pool.tile([128, 512], f32, tag="sbc",
                                       name=f"sbc{ch}")
                nc.sync.dma_start(
                    sbc[:], s_dram[:].flatten()[cs].partition_broadcast(128))
                for ct in range(2):
                    nc.vector.tensor_tensor(post_raw[ct][:, cs],
                                            post_raw[ct][:, cs],
                                            sbc[:], op=Alu.mult)
                    nc.scalar.copy(post_hi[ct][:, cs], post_raw[ct][:, cs])
                    nc.vector.tensor_tensor(post_lo[ct][:, cs],
                                            post_raw[ct][:, cs],
                                            post_hi[ct][:, cs], op=Alu.subtract)

        # ---------------- steady loop over query tiles ----------------
        loop = ctx.enter_context(tc.tile_pool(name="loop_sb", bufs=2))
        loop3 = ctx.enter_context(tc.tile_pool(name="loop3_sb", bufs=3))
        at_pool = ctx.enter_context(tc.tile_pool(name="at_sb", bufs=4))
        cands_pool = ctx.enter_context(tc.tile_pool(name="cands_sb", bufs=2))
        ps_sim = ctx.enter_context(tc.tile_pool(name="ps_sim", bufs=6, space="PSUM"))
        ps_oe = ctx.enter_context(tc.tile_pool(name="ps_oe", bufs=2, space="PSUM"))


        for rep in range(reps):
          for T in range(NT):
            simT = loop.tile([128, N], f32, tag="simT")
            qs = slice(T * 128, (T + 1) * 128)
            passes = [(post_hi[0], post_hi[0]), (post_hi[0], post_lo[0]),
                      (post_lo[0], post_hi[0]), (post_hi[1], post_hi[1]),
                      (post_hi[1], post_lo[1]), (post_lo[1], post_hi[1])]
            for half in range(2):
                sms = [ps_sim.tile([128, 512], f32, tag="sm",
                                   name=f"sm{T}_{half}_{i}") for i in range(4)]
                for pi, (lhs_t, rhs_t) in enumerate(passes):
                    for r in range(4):
                        o = half * 2048 + r * 512
                        nc.tensor.matmul(sms[r][:],
                                         lhs_t[:, qs], rhs_t[:, o:o + 512],
                                         start=(pi == 0), stop=(pi == 5))
                for r in range(4):
                    nc.scalar.copy(
                        simT[:, half * 2048 + r * 512:half * 2048 + (r + 1) * 512],
                        sms[r][:])

            # selection: 32-chunk max8 -> 256 candidates -> exact top-32 value
            cands = cands_pool.tile([128, 256], f32, tag="cands")
            for c in range(32):
                nc.vector.max(cands[:, c * 8:(c + 1) * 8],
                              simT[:, c * 128:(c + 1) * 128])
            r8 = cands_pool.tile([128, 8], f32, tag="r8")
            for rnd in range(4):
                nc.vector.max(r8[:], cands[:])
                if rnd < 3:
                    nc.vector.match_replace(out=cands[:], in_to_replace=r8[:],
                                            in_values=cands[:], imm_value=NEG)
            # masked exp weights: mask = (sim >= v32) exactly, on DVE at 2x
            E = loop3.tile([128, N], f16, tag="E")
            M = loop3.tile([128, N], f16, tag="M")
            nc.scalar.activation(E[:], simT[:], AF.Exp)
            nc.vector.tensor_scalar(M[:], simT[:], r8[:, 7:8], scalar2=None,
                                    op0=Alu.is_ge)
            A = M
            nc.vector.tensor_tensor(A[:], E[:], M[:], op=Alu.mult)

            # blocked transpose via DRAM round-trip
            AT = at_pool.tile([128, NT, 128], f16, tag="AT")
            nc.sync.dma_start(a_drams[T % 3][:], A[:])
            nc.sync.dma_start_transpose(AT[:], a_drams[T % 3][:])

            # gather matmul: out_ext = A @ [feat | 1 | 0]
            oe = ps_oe.tile([128, C + 2], f32, tag="oe")
            for j in range(NT):
                nc.tensor.matmul(oe[:], AT[:, j, :], featx[j][:],
                                 start=(j == 0), stop=(j == NT - 1))
            rz = cands_pool.tile([128, 1], f32, tag="rz")
            nc.vector.reciprocal(rz[:], oe[:, C:C + 1])
            osb = loop.tile([128, C], f32, tag="osb")
            nc.scalar.activation(osb[:], oe[:, 0:C], AF.Copy, scale=rz[:])
            nc.sync.dma_start(out_p[T * 128:(T + 1) * 128, :], osb[:])

    nc.compile()
    return nc


def kernel(x, W, bias, k):
    from concourse.bass_utils import run_bass_kernel_spmd

    x = np.asarray(x, dtype=np.float32)
    W = np.asarray(W, dtype=np.float32)
    bias = np.asarray(bias, dtype=np.float32)
    assert int(k) == K and x.shape == (B, N, C)

    if "nc" not in _CACHE:
        _CACHE["nc"] = _build()
    nc = _CACHE["nc"]

    wT = np.ascontiguousarray(W.T)                      # [C, 2C]
    bf = bias.reshape(1, 2 * C)
    bp = np.ascontiguousarray(
        bias[C:].reshape(2, 128).T)                     # [128, 2]
    ident = np.eye(128, dtype=np.float32)

    in_maps = []
    for b in range(B):
        xT = np.ascontiguousarray(x[b].T)               # [C, N]
        in_maps.append({"xT": xT, "wT": wT, "bf": bf, "bp": bp, "ident": ident})

    res = run_bass_kernel_spmd(nc, in_maps, list(range(B)))
    out = np.stack([res.results[b]["out"] for b in range(B)], axis=0)
    return out.astype(np.float32)

